# revision 28
# baseline (speedup 1.0000x reference)
"""Trainium2 Bass kernel for AccumulativeGainLoss.

Data-parallel over B across 8 NeuronCores (2 batch elements per core).

Math restructuring (validated to ~2.5e-6 rel err in f32 vs the jax reference):
for each batch element b, with F = preds[b] [N,K] and Y = y_ts[b] rearranged
to [N, T*D]:
    H   = [F|1]^T [F|1]                 (Gram + column sums + N)
    inv = (F^T F)^{-1}                  (Newton-Schulz, 5 iters, X0=(K/tr)I)
    M   = F^T Y, sumy = 1^T Y, sy2 = 1^T (Y*Y)
    q   = colsum(M * (inv M))           (= diag(M^T inv M))
    ss_res = sy2 - q                    (beta^T FtF beta ≈ beta^T M, err ~1e-12)
    ss_tot = sy2 - sumy^2/N + EPS
    r2  = 1 - ss_res/ss_tot ;  wsum_b = sum_td w[t,d] * r2[t,d]
    cov = FtF - s s^T / N ; c = 1/diag(cov) ; quad_b = c^T (cov*cov) c
loss = mean_b( -wsum_b/T ) + 0.1 * mean_b( quad_b - K )

The big tensor Y (12.3 MB/core) is streamed through SBUF once; all
reductions over N happen on the TensorEngine (PSUM accumulation over
47 chunks of 128 rows). sy2 needs Y^2, computed on ScalarE.
"""

import ml_dtypes
import numpy as np

import concourse.bacc as bacc
import concourse.bass as bass
import concourse.mybir as mybir
import concourse.tile as tile
from concourse.bass_utils import run_bass_kernel_spmd

F32 = mybir.dt.float32
BF16 = mybir.dt.bfloat16
ALU = mybir.AluOpType
AX = mybir.AxisListType

B, T, N, K, D = 16, 32, 6000, 32, 8
NCORES = 8
JB = B // NCORES          # batch elements per core
NCH = 47                  # ceil(6000/128) chunks of 128 rows
NPAD = NCH * 128          # 6016
TD = T * D                # 256
FW = 34                   # per-chunk F block: 32 coeffs + ones col + pad
FROW = NCH * FW           # 1598
YROW = NCH * TD           # 12032
BLOCKS = (12, 12, 12, 11)  # chunk blocking of the Y stream
NS_ITERS = 5
EPS = 1e-8
DECAY = 0.9
PEN = 0.1

_CACHE = {}


def _build_program():
    nc = bacc.Bacc("TRN2", target_bir_lowering=False, debug=False)
    y_d = nc.declare_dram_parameter("y", [JB, 128, YROW], BF16, isOutput=False)
    f_d = nc.declare_dram_parameter("f", [JB, 128, FROW], BF16, isOutput=False)
    c_d = nc.declare_dram_parameter("c32", [32, 96], F32, isOutput=False)
    w_d = nc.declare_dram_parameter("w2", [1, TD], F32, isOutput=False)
    o_d = nc.declare_dram_parameter("out", [1, 2], F32, isOutput=True)

    with tile.TileContext(nc) as tc:
        with (
            tc.tile_pool(name="cpool", bufs=1) as cpool,
            tc.tile_pool(name="fpool", bufs=1) as fpool,
            tc.tile_pool(name="ypool", bufs=8) as ypool,
            tc.tile_pool(name="sqpool", bufs=8) as sqpool,
            tc.tile_pool(name="nsb", bufs=2) as nsb,
            tc.tile_pool(name="esb", bufs=2) as esb,
            # PSUM is exactly 8 banks; every tag below gets one bank.
            tc.tile_pool(name="ps", bufs=1, space="PSUM") as ps,
        ):
            # constants come in via DMA, then bounce through a DVE copy so
            # downstream TensorTensor ops depend on DVE FIFO order only —
            # walrus's TT encoding has room for a single sync wait.
            consts0 = cpool.tile([32, 96], F32)
            nc.sync.dma_start(out=consts0, in_=c_d[:, :])
            consts = cpool.tile([32, 96], F32)
            nc.vector.tensor_copy(consts, consts0)
            eye = consts[:, 0:32]
            twoI = consts[:, 32:64]
            ones2d = consts[:, 64:96]
            ones32 = consts[:, 64:65]

            w2sb0 = cpool.tile([1, TD], F32)
            nc.sync.dma_start(out=w2sb0, in_=w_d[:, :])
            w2sb = cpool.tile([1, TD], F32)
            nc.vector.tensor_copy(w2sb, w2sb0)

            ftile = fpool.tile([128, JB * FROW], BF16)
            for j in range(JB):
                nc.sync.dma_start(
                    out=ftile[:, j * FROW:(j + 1) * FROW], in_=f_d[j, :, :]
                )

            def fch(j, c):  # chunk-c F block [128, 33] (coeffs + ones)
                return ftile[:, j * FROW + c * FW: j * FROW + c * FW + 33]

            def fones(j, c):  # chunk-c ones column (0 in padded rows)
                return ftile[:, j * FROW + c * FW + 32: j * FROW + c * FW + 33]

            # ---- Newton-Schulz inverse of FtF + correlation penalty, per j
            inv_sb = []
            quad_sb = []
            for j in range(JB):
                # Gram matrix H_j = [F|1]^T [F|1] (accumulated over chunks)
                Hps = ps.tile([33, 33], F32, tag=f"H{j}")
                for c in range(NCH):
                    nc.tensor.matmul(
                        Hps, fch(j, c), fch(j, c),
                        start=(c == 0), stop=(c == NCH - 1),
                    )
                Hsb = nsb.tile([33, 33], F32, tag="Hsb")
                nc.vector.tensor_copy(Hsb, Hps)
                A = Hsb[0:32, 0:32]
                s_row = Hsb[32:33, 0:32]

                # trace of A, broadcast to all 32 partitions
                dm = nsb.tile([32, 32], F32, tag="dm")
                nc.vector.tensor_mul(dm, A, eye)
                dg = nsb.tile([32, 1], F32, tag="dg")
                nc.vector.reduce_sum(dg, dm, axis=AX.X)
                trp = ps.tile([32, 32], F32, tag="tns")
                nc.tensor.matmul(trp[:, 0:1], ones2d, dg, start=True, stop=True)
                rtr = nsb.tile([32, 1], F32, tag="rtr")
                nc.vector.reciprocal(rtr, trp[:, 0:1])
                c0v = nsb.tile([32, 1], F32, tag="c0v")
                nc.vector.tensor_scalar_mul(c0v, rtr, float(K))
                X = nsb.tile([32, 32], F32, tag="Xns", bufs=NS_ITERS + 2)
                nc.vector.tensor_scalar(X, eye, c0v, None, ALU.mult)

                for _ in range(NS_ITERS):
                    t1 = ps.tile([32, 32], F32, tag="tns")
                    nc.tensor.matmul(t1, A, X, start=True, stop=True)
                    z = nsb.tile([32, 32], F32, tag="Zns", bufs=NS_ITERS + 1)
                    nc.vector.tensor_sub(z, twoI, t1)
                    x2 = ps.tile([32, 32], F32, tag="tns")
                    nc.tensor.matmul(x2, X, z, start=True, stop=True)
                    Xn = nsb.tile([32, 32], F32, tag="Xns", bufs=NS_ITERS + 2)
                    nc.vector.tensor_copy(Xn, x2)
                    X = Xn
                inv_sb.append(X)

                # correlation penalty: quad = c^T (cov*cov) c, c = 1/diag(cov)
                outp = ps.tile([32, 32], F32, tag="tns")
                nc.tensor.matmul(outp, s_row, s_row, start=True, stop=True)
                covn = nsb.tile([32, 32], F32, tag="covn")
                nc.vector.tensor_scalar_mul(covn, outp, 1.0 / N)
                cov = nsb.tile([32, 32], F32, tag="cov")
                nc.vector.tensor_sub(cov, A, covn)
                dm2 = nsb.tile([32, 32], F32, tag="dm2")
                nc.vector.tensor_mul(dm2, cov, eye)
                dg2 = nsb.tile([32, 1], F32, tag="dg2")
                nc.vector.reduce_sum(dg2, dm2, axis=AX.X)
                cv = nsb.tile([32, 1], F32, tag="cv")
                nc.vector.reciprocal(cv, dg2)
                A2 = nsb.tile([32, 32], F32, tag="A2")
                nc.vector.tensor_mul(A2, cov, cov)
                ups = ps.tile([32, 32], F32, tag="tns")
                nc.tensor.matmul(ups[:, 0:1], A2, cv, start=True, stop=True)
                usb = nsb.tile([32, 1], F32, tag="usb")
                nc.vector.tensor_copy(usb, ups[:, 0:1])
                qd = ps.tile([32, 32], F32, tag="tns")
                nc.tensor.matmul(qd[0:1, 0:1], usb, cv, start=True, stop=True)
                qsb = nsb.tile([1, 1], F32, tag="qsb")
                nc.vector.tensor_copy(qsb, qd[0:1, 0:1])
                quad_sb.append(qsb)

            # results staging: [wsum0, wsum1, quad0, quad1]
            wsout = cpool.tile([1, 4], F32)

            # ---- stream Y, accumulate G = [F|1]^T Y and sy2 = 1^T Y^2
            for j in range(JB):
                Gps = ps.tile([33, TD], F32, tag=f"G{j}")
                Sps = ps.tile([1, TD], F32, tag=f"S{j}")
                c0 = 0
                for blk in BLOCKS:
                    # Fresh SBUF slots for every block (bufs=8, no reuse):
                    # each engine instruction then needs at most ONE sync
                    # wait, which is all the 64B encoding has room for
                    # (and Tile's redundant-wait eliminator is disabled).
                    yt = ypool.tile([128, blk * TD], BF16, tag="yt")
                    nc.sync.dma_start(
                        out=yt, in_=y_d[j, :, c0 * TD:(c0 + blk) * TD]
                    )
                    sq = sqpool.tile([128, blk * TD], BF16, tag="sq")
                    nc.scalar.square(sq, yt)
                    for lc in range(blk):
                        c = c0 + lc
                        nc.tensor.matmul(
                            Gps, fch(j, c), yt[:, lc * TD:(lc + 1) * TD],
                            start=(c == 0), stop=(c == NCH - 1),
                        )
                        nc.tensor.matmul(
                            Sps, fones(j, c), sq[:, lc * TD:(lc + 1) * TD],
                            start=(c == 0), stop=(c == NCH - 1),
                        )
                    c0 += blk

                # ---- per-j epilogue
                Gsb = esb.tile([33, TD], F32, tag="Gsb")
                nc.vector.tensor_copy(Gsb, Gps)
                M = Gsb[0:32, :]
                sumy = Gsb[32:33, :]

                sy2sb = esb.tile([1, TD], F32, tag="sy2sb")
                nc.vector.tensor_copy(sy2sb, Sps)

                Pps = ps.tile([32, TD], F32, tag="tPq")
                nc.tensor.matmul(Pps, inv_sb[j], M, start=True, stop=True)
                # bounce P through SBUF: the copy carries the single PE wait,
                # the 3-source multiply then sees only same-engine operands
                Psb = esb.tile([32, TD], F32, tag="Psb")
                nc.vector.tensor_copy(Psb, Pps)
                W = esb.tile([32, TD], F32, tag="W")
                nc.vector.tensor_mul(W, M, Psb)
                qps = ps.tile([32, TD], F32, tag="tPq")
                nc.tensor.matmul(qps[0:1, :], ones32, W, start=True, stop=True)
                qsb = esb.tile([1, TD], F32, tag="qsb2")
                nc.vector.tensor_copy(qsb, qps[0:1, :])
                ssres = esb.tile([1, TD], F32, tag="ssres")
                nc.vector.tensor_sub(ssres, sy2sb, qsb)
                sumy2 = esb.tile([1, TD], F32, tag="sumy2")
                nc.vector.tensor_mul(sumy2, sumy, sumy)
                sstot_a = esb.tile([1, TD], F32, tag="sstot_a")
                nc.vector.tensor_scalar(
                    sstot_a, sumy2, -1.0 / N, EPS, ALU.mult, ALU.add
                )
                sstot = esb.tile([1, TD], F32, tag="sstot")
                nc.vector.tensor_add(sstot, sstot_a, sy2sb)
                rec = esb.tile([1, TD], F32, tag="rec")
                nc.vector.reciprocal(rec, sstot)
                ratio = esb.tile([1, TD], F32, tag="ratio")
                nc.vector.tensor_mul(ratio, ssres, rec)
                r2 = esb.tile([1, TD], F32, tag="r2")
                nc.vector.tensor_scalar(r2, ratio, -1.0, 1.0, ALU.mult, ALU.add)
                scratch = esb.tile([1, TD], F32, tag="scratch")
                nc.vector.tensor_mul(scratch, r2, w2sb)
                nc.vector.reduce_sum(wsout[:, j:j + 1], scratch, axis=AX.X)
                nc.vector.tensor_copy(wsout[:, 2 + j:3 + j], quad_sb[j])

            outsb = cpool.tile([1, 2], F32)
            nc.vector.tensor_add(outsb[:, 0:1], wsout[:, 0:1], wsout[:, 1:2])
            nc.vector.tensor_add(outsb[:, 1:2], wsout[:, 2:3], wsout[:, 3:4])
            nc.gpsimd.dma_start(out=o_d[:, :], in_=outsb)

    nc.compile()
    return nc


def _prepare_in_maps(preds, y_ts, importance):
    preds = np.ascontiguousarray(preds, dtype=np.float32)
    y_ts = np.ascontiguousarray(y_ts, dtype=np.float32)
    importance = np.ascontiguousarray(importance, dtype=np.float32)

    bf16 = ml_dtypes.bfloat16

    # Y image: yimg[b, p, c*TD + t*D + d] = y_ts[b, t, c*128+p, d]
    ypad = np.zeros((B, T, NPAD, D), dtype=bf16)
    ypad[:, :, :N, :] = y_ts.astype(bf16)
    yimg = np.ascontiguousarray(
        ypad.reshape(B, T, NCH, 128, D).transpose(0, 3, 2, 1, 4)
    ).reshape(B, 128, YROW)

    # F image: fimg[b, p, c*FW + k] = preds[b, c*128+p, k]; col 32 = valid-mask
    fpad = np.zeros((B, NPAD, FW), dtype=bf16)
    fpad[:, :N, :K] = preds.astype(bf16)
    fpad[:, :N, K] = 1.0
    fimg = np.ascontiguousarray(
        fpad.reshape(B, NCH, 128, FW).transpose(0, 2, 1, 3)
    ).reshape(B, 128, FROW)

    c32 = np.zeros((32, 96), dtype=np.float32)
    c32[:, 0:32] = np.eye(32, dtype=np.float32)
    c32[:, 32:64] = 2.0 * np.eye(32, dtype=np.float32)
    c32[:, 64:96] = 1.0

    decay = DECAY ** np.arange(T, dtype=np.float32)
    w2 = (decay[:, None] * importance[None, :].astype(np.float32)).reshape(1, TD)
    w2 = np.ascontiguousarray(w2, dtype=np.float32)

    in_maps = []
    for i in range(NCORES):
        in_maps.append({
            "y": np.ascontiguousarray(yimg[i * JB:(i + 1) * JB]),
            "f": np.ascontiguousarray(fimg[i * JB:(i + 1) * JB]),
            "c32": c32,
            "w2": w2,
        })
    return in_maps


def _combine(results):
    loss = 0.0
    for r in results:
        w_total, q_total = float(r["out"][0, 0]), float(r["out"][0, 1])
        loss += (-w_total / T + PEN * (q_total - JB * K)) / B
    return np.float32(loss)


def run_on_device(preds, y_ts, importance, trace=False, **spmd_kwargs):
    if "nc" not in _CACHE:
        _CACHE["nc"] = _build_program()
    nc = _CACHE["nc"]
    in_maps = _prepare_in_maps(preds, y_ts, importance)
    res = run_bass_kernel_spmd(
        nc, in_maps, list(range(NCORES)), trace=trace, **spmd_kwargs
    )
    return _combine(res.results), res


def kernel(preds, y_ts, importance):
    loss, _ = run_on_device(preds, y_ts, importance, trace=False)
    return loss
